# revision 21
# baseline (speedup 1.0000x reference)
"""Trainium2 Bass kernel: Conformer MHSA with relative positional encoding.

Shapes (hardcoded): B=8, T=1024, E=512, H=8, DH=64.
Sharding: data-parallel over batch -- one batch element per NeuronCore (8 cores).

Math notes (validated against the reference in numpy):
  - LayerNorm gamma/beta are folded into the projection weights/biases on host.
  - The relative-position term bd[i,j] = q_v[i] . pe(i-j) is decomposed via
    angle-addition into A(i).cos(a j) + B(i).sin(a j), where
      S = (q_v)*sin(a i)  (per f-dim, rows 0:64 hold both qs- and qc-halves)
      C = (q_v)*cos(a i)
      A = S[qs] + C[qc],  B = S[qc] - C[qs]
    The A/B row combination is one PE matmul with a constant +-1 matrix (Mc)
    for the C-part plus a DVE tensor_add for the S-part, exploiting that
    compute engines CAN write to a different (quadrant-aligned) partition
    base than they read -- no DMA partition-shifts anywhere in this kernel.
  - k-projection bias dropped (softmax-invariant); v bias folded into the
    output-projection bias.
  - Softmax skips max-subtraction; the denominator rides the AV matmul as
    mask-columns (M=128: 64 v-cols + 64 mask-cols) and divides the small
    (64,T) per-head output.
  - P (softmax numerator), V, oT and the output weights are bf16 (measured
    full-pipeline rel err 3e-3 vs the 2e-2 gate); everything else f32/f32r.
"""

import os
import sys

import numpy as np

sys.path.insert(0, "/opt/trn_rl_repo")

T = 1024
E = 512
H = 8
DH = 64
NT = T // 128  # 8 i/j tiles
NC = E // 128  # 4 c/e tiles
LN_EPS = 1e-5
N_CORES = 8

_CACHE = {}


def _build_nc():
    import concourse.bass as bass
    import concourse.tile as tile
    from concourse import bacc, mybir

    f32 = mybir.dt.float32
    f32r = mybir.dt.float32r
    bf16 = mybir.dt.bfloat16
    Alu = mybir.AluOpType
    Act = mybir.ActivationFunctionType

    def r(ap):
        return ap.bitcast(f32r)

    nc = bacc.Bacc("TRN2", target_bir_lowering=False, debug=False, num_devices=N_CORES)

    x_d = nc.declare_dram_parameter("x", [T, E], f32, isOutput=False)
    qwT_d = nc.declare_dram_parameter("qwT", [E, E], bf16, isOutput=False)
    kwT_d = nc.declare_dram_parameter("kwT", [E, E], bf16, isOutput=False)
    vwT_d = nc.declare_dram_parameter("vwT", [E, E], bf16, isOutput=False)
    owT_d = nc.declare_dram_parameter("owT", [E, E], bf16, isOutput=False)
    pb_d = nc.declare_dram_parameter("pbias", [128, 8], f32, isOutput=False)
    tblS_d = nc.declare_dram_parameter("tblsin", [128, T], f32, isOutput=False)
    tblC_d = nc.declare_dram_parameter("tblcos", [128, T], f32, isOutput=False)
    tabk_d = nc.declare_dram_parameter("tabk", [64, T], f32r, isOutput=False)
    ob_d = nc.declare_dram_parameter("obias", [128, E], f32, isOutput=False)
    mask_d = nc.declare_dram_parameter("maskt", [128, NT], f32, isOutput=False)
    id_d = nc.declare_dram_parameter("ident", [128, 128], f32r, isOutput=False)
    mc_d = nc.declare_dram_parameter("mcomb", [128, 128], f32r, isOutput=False)
    out_d = nc.declare_dram_parameter("out", [T, E], f32, isOutput=True)

    # [128, t, e] views of x / out (t-major row blocks on partitions)
    x_v = x_d[:].rearrange("(t p) e -> p t e", p=128)
    out_v = out_d[:].rearrange("(t p) e -> p t e", p=128)

    with tile.TileContext(nc) as tc:
        from contextlib import ExitStack

        with ExitStack() as ctx:
            consts = ctx.enter_context(tc.tile_pool(name="consts", bufs=1))
            sb = ctx.enter_context(tc.tile_pool(name="sb", bufs=1))
            ps = ctx.enter_context(tc.tile_pool(name="ps", bufs=1, space="PSUM"))

            # ---- tiles ----
            wq = consts.tile([128, NC * 512], bf16, tag="wq")
            wk = consts.tile([128, NC * 512], bf16, tag="wk")
            wv = consts.tile([128, NC * 512], bf16, tag="wv")
            wo = consts.tile([128, NC * 512], bf16, tag="wo")
            tblS = consts.tile([128, T], f32, tag="tblS")
            tblC = consts.tile([128, T], f32, tag="tblC")
            tabk = consts.tile([128, T], f32r, tag="tabk")
            pb = consts.tile([128, 8], f32, tag="pb")
            ob = consts.tile([128, E], f32, tag="ob")
            mk = consts.tile([128, NT], f32, tag="mk")
            ident = consts.tile([128, 128], f32r, tag="ident")
            mcomb = consts.tile([128, 128], f32r, tag="mcomb")
            epsc = consts.tile([128, 1], f32, tag="epsc")
            nc.vector.memset(epsc[:], LN_EPS)
            ones = consts.tile([128, 512], f32, tag="ones")
            nc.vector.memset(ones[:], 1.0)
            kbig = consts.tile([128, H * T], f32r, tag="kbig")  # rows 64:128 = tables
            qbig = consts.tile([128, H * T], f32r, tag="qbig")
            zT = consts.tile([128, NC * T], bf16, tag="zT")
            vx = consts.tile([128, NT * 1024], bf16, tag="vx")
            oT = [
                consts.tile([128, T], bf16, tag=f"oT{i}", name=f"oT{i}")
                for i in range(NC)
            ]

            # ---- DMAs, ordered by first use: x gates LN, so it goes first;
            # wo/ob aren't needed until the output projection at the very end.
            xts = []
            for g in range(2):
                xt = sb.tile([128, 4 * E], f32, tag="x", bufs=2, name=f"xt{g}")
                xts.append(xt)
                nc.sync.dma_start(
                    xt[:].rearrange("p (t e) -> p t e", e=E),
                    x_v[:, g * 4 : (g + 1) * 4, :],
                )
                if g == 0:
                    nc.sync.dma_start(ident[:], id_d[:])
            nc.sync.dma_start(
                wq[:].rearrange("p (c e) -> p c e", e=512),
                qwT_d[:].rearrange("(c p) e -> p c e", p=128),
            )
            nc.sync.dma_start(tblS[:], tblS_d[:])
            nc.sync.dma_start(tblC[:], tblC_d[:])
            nc.sync.dma_start(pb[:], pb_d[:])
            nc.sync.dma_start(mcomb[:], mc_d[:])
            nc.sync.dma_start(
                wk[:].rearrange("p (c e) -> p c e", e=512),
                kwT_d[:].rearrange("(c p) e -> p c e", p=128),
            )
            nc.sync.dma_start(tabk[64:128, :], tabk_d[:])
            nc.sync.dma_start(
                wv[:].rearrange("p (c e) -> p c e", e=512),
                vwT_d[:].rearrange("(c p) e -> p c e", p=128),
            )
            nc.sync.dma_start(mk[:], mask_d[:])
            nc.sync.dma_start(
                wo[:].rearrange("p (c e) -> p c e", e=512),
                owT_d[:].rearrange("(c p) e -> p c e", p=128),
            )
            nc.sync.dma_start(ob[:], ob_d[:])

            # ---- phase A: LayerNorm + transpose ----
            for g in range(2):
                xt = xts[g]
                for tt in range(4):
                    t = g * 4 + tt
                    xs = xt[:, tt * E : (tt + 1) * E]
                    st = sb.tile([128, 6], f32, tag="st", bufs=2)
                    nc.vector.bn_stats(st[:], xs)
                    mv = sb.tile([128, 2], f32, tag="mv", bufs=2)
                    nc.vector.bn_aggr(mv[:], st[:])
                    sd = sb.tile([128, 1], f32, tag="sd", bufs=2)
                    nc.scalar.activation(sd[:], mv[:, 1:2], Act.Sqrt, bias=epsc[:], scale=1.0)
                    rstd = sb.tile([128, 1], f32, tag="rstd", bufs=4)
                    nc.vector.reciprocal(rstd[:], sd[:])
                    nmr = sb.tile([128, 1], f32, tag="nmr", bufs=4)
                    nc.vector.scalar_tensor_tensor(
                        nmr[:], mv[:, 0:1], -1.0, rstd[:], Alu.mult, Alu.mult
                    )
                    zt = sb.tile([128, E], f32r, tag="z", bufs=2)
                    nc.scalar.activation(zt[:], xs, Act.Identity, bias=nmr[:], scale=rstd[:])
                    # transpose: 4 c-blocks into one psum tile, one strided evac
                    pt = ps.tile([128, T], f32, tag="pA", bufs=2)
                    for c in range(NC):
                        nc.tensor.transpose(
                            r(pt[:, c * 128 : (c + 1) * 128]),
                            zt[:, c * 128 : (c + 1) * 128],
                            ident[:],
                        )
                    nc.vector.tensor_copy(
                        zT[:].rearrange("p (c i) -> p c i", i=T)[:, :, t * 128 : (t + 1) * 128],
                        pt[:, 0:512].rearrange("p (c i) -> p c i", i=128),
                    )

            # ---- phase B: Q/K projections + rel-pos prep, per e-tile (2 heads)
            def emit_qk(et):
                h0, h1 = 2 * et, 2 * et + 1
                # Q^T projection: [128 e', T]
                psq = ps.tile([128, T], f32, tag="pA", bufs=2, name=f"psq{et}")
                for ic in range(2):
                    for c in range(NC):
                        nc.tensor.matmul(
                            psq[:, ic * 512 : (ic + 1) * 512],
                            wq[:, c * 512 + et * 128 : c * 512 + (et + 1) * 128],
                            zT[:, c * T + ic * 512 : c * T + (ic + 1) * 512],
                            start=(c == 0),
                            stop=(c == NC - 1),
                        )
                # S/C = (q + b_v) * sin/cos tables -- first in the DVE stream,
                # they gate the pc matmul
                S = sb.tile([128, T], f32r, tag="S", bufs=1, name=f"S{et}")
                nc.vector.scalar_tensor_tensor(
                    S[:], psq[:], pb[:, 4 + et : 5 + et], tblS[:], Alu.add, Alu.mult
                )
                C = sb.tile([128, T], f32r, tag="C", bufs=1, name=f"C{et}")
                nc.vector.scalar_tensor_tensor(
                    C[:], psq[:], pb[:, 4 + et : 5 + et], tblC[:], Alu.add, Alu.mult
                )
                # q_u rows on ACT (runs parallel to the DVE chain)
                nc.scalar.activation(
                    qbig[0:64, h0 * T : (h0 + 1) * T],
                    psq[0:64, :],
                    Act.Identity,
                    bias=pb[0:64, et : et + 1],
                    scale=1.0,
                )
                nc.scalar.activation(
                    qbig[0:64, h1 * T : (h1 + 1) * T],
                    psq[64:128, :],
                    Act.Identity,
                    bias=pb[64:128, et : et + 1],
                    scale=1.0,
                )
                # K^T projection
                psk = ps.tile([128, T], f32, tag="pA", bufs=2, name=f"psk{et}")
                for ic in range(2):
                    for c in range(NC):
                        nc.tensor.matmul(
                            psk[:, ic * 512 : (ic + 1) * 512],
                            wk[:, c * 512 + et * 128 : c * 512 + (et + 1) * 128],
                            zT[:, c * T + ic * 512 : c * T + (ic + 1) * 512],
                            start=(c == 0),
                            stop=(c == NC - 1),
                        )
                # pc = Mc . C  (the +-1 row-combination of the C-part)
                pc = ps.tile([128, T], f32, tag="pA", bufs=2, name=f"pc{et}")
                for ic in range(2):
                    nc.tensor.matmul(
                        pc[:, ic * 512 : (ic + 1) * 512],
                        mcomb[:],
                        C[:, ic * 512 : (ic + 1) * 512],
                        start=True,
                        stop=True,
                    )
                # qbig rows 64:128 = [A;B] = pc + S  (quadrant-shifted adds)
                nc.vector.tensor_add(
                    qbig[64:128, h0 * T : (h0 + 1) * T],
                    pc[0:64, :],
                    S[0:64, :],
                )
                nc.vector.tensor_add(
                    qbig[64:128, h1 * T : (h1 + 1) * T],
                    pc[64:128, :],
                    S[64:128, :],
                )
                # k rows + this head-pair's position-table rows of kbig
                nc.vector.tensor_copy(
                    kbig[0:64, h0 * T : (h0 + 1) * T], psk[0:64, :]
                )
                nc.scalar.copy(
                    kbig[0:64, h1 * T : (h1 + 1) * T], psk[64:128, :]
                )
                nc.vector.tensor_copy(
                    kbig[64:128, h0 * T : (h0 + 1) * T], tabk[64:128, :]
                )
                nc.vector.tensor_copy(
                    kbig[64:128, h1 * T : (h1 + 1) * T], tabk[64:128, :]
                )
            # position-table rows of kbig (only depend on the tabk DMA;
            # fills the early DVE/ACT gaps)
            for h in range(H):
                if h % 2 == 0:
                    nc.vector.tensor_copy(
                        kbig[64:128, h * T : (h + 1) * T], tabk[64:128, :]
                    )
                else:
                    nc.scalar.copy(
                        kbig[64:128, h * T : (h + 1) * T], tabk[64:128, :]
                    )

            # ---- phase C: V projection (natural layout) + mask columns ----
            def emit_v(jt):
                psv = ps.tile([128, T], f32, tag="pA", bufs=2, name=f"psv{jt}")
                for c in range(NC):
                    nc.tensor.matmul(
                        psv[:, 0:512],
                        zT[:, c * T + jt * 128 : c * T + (jt + 1) * 128],
                        wv[:, c * 512 : (c + 1) * 512],
                        start=(c == 0),
                        stop=(c == NC - 1),
                    )
                vx3 = vx[:, jt * 1024 : (jt + 1) * 1024].rearrange(
                    "p (h f) -> p h f", f=128
                )
                # v columns (masked, cast to bf16; scale is per-partition = per-j)
                nc.vector.tensor_scalar(
                    vx3[:, :, 0:DH],
                    psv[:, 0:512].rearrange("p (h f) -> p h f", f=DH),
                    mk[:, jt : jt + 1],
                    None,
                    Alu.mult,
                )
                # mask columns (denominator rows of the AV matmul)
                nc.vector.tensor_scalar(
                    vx3[:, :, DH:128],
                    ones[:].rearrange("p (h f) -> p h f", f=DH),
                    mk[:, jt : jt + 1],
                    None,
                    Alu.mult,
                )

            p_tiles = {}

            def emit_scores(h):
                tiles = []
                for jt in range(NT):
                    psl = ps.tile([128, T], f32, tag="pA", bufs=2, name=f"psl{h}_{jt}")
                    for ic in range(2):
                        nc.tensor.matmul(
                            psl[:, ic * 512 : (ic + 1) * 512],
                            kbig[:, h * T + jt * 128 : h * T + (jt + 1) * 128],
                            qbig[:, h * T + ic * 512 : h * T + (ic + 1) * 512],
                            start=True,
                            stop=True,
                        )
                    pexp = sb.tile([128, T], bf16, tag="P", bufs=15)
                    tiles.append(pexp)
                    nc.scalar.activation(pexp[:], psl[:], Act.Exp, scale=0.125)
                p_tiles[h] = tiles

            def emit_av(h):
                tiles = p_tiles.pop(h)
                psav = ps.tile([128, T], f32, tag="pB", bufs=2)
                for jt in range(NT):
                    for ic in range(2):
                        nc.tensor.matmul(
                            psav[:, ic * 512 : (ic + 1) * 512],
                            vx[:, jt * 1024 + h * 128 : jt * 1024 + (h + 1) * 128],
                            tiles[jt][:, ic * 512 : (ic + 1) * 512],
                            start=(jt == 0),
                            stop=(jt == NT - 1),
                        )
                # denominator (rows 64:128 hold it replicated), divide into oT
                rr = sb.tile([128, T], f32, tag="rr", bufs=1)
                nc.vector.reciprocal(rr[64:128, :], psav[64:128, :])
                rows = slice(0, DH) if h % 2 == 0 else slice(DH, 2 * DH)
                nc.vector.tensor_mul(
                    oT[h // 2][rows, :], psav[0:DH, :], rr[64:128, :]
                )

            emit_qk(0)
            emit_qk(1)
            for jt in range(NT):
                emit_v(jt)
            emit_scores(0)
            emit_scores(1)
            emit_av(0)
            emit_scores(2)
            emit_qk(2)
            emit_av(1)
            emit_scores(3)
            emit_av(2)
            emit_scores(4)
            emit_qk(3)
            emit_av(3)

            emit_scores(5)
            emit_av(4)
            emit_scores(6)
            emit_av(5)
            emit_scores(7)
            emit_av(6)
            emit_av(7)

            # ---- phase E: output projection (pairs of i-tiles per store) ----
            for u in range(NT // 2):
                yt = sb.tile([128, 2 * E], f32, tag="y", bufs=2)
                for half in range(2):
                    it = 2 * u + half
                    psy = ps.tile([128, T], f32, tag="pA", bufs=2)
                    for ft in range(NC):
                        nc.tensor.matmul(
                            psy[:, 0:512],
                            oT[ft][:, it * 128 : (it + 1) * 128],
                            wo[:, ft * 512 : (ft + 1) * 512],
                            start=(ft == 0),
                            stop=(ft == NC - 1),
                        )
                    nc.vector.tensor_add(
                        yt[:, half * E : (half + 1) * E], psy[:, 0:512], ob[:]
                    )
                nc.sync.dma_start(
                    out_v[:, 2 * u : 2 * u + 2, :],
                    yt[:].rearrange("p (t e) -> p t e", e=E),
                )

    if not nc.is_finalized():
        nc.finalize()
    return nc


def _host_prep(inputs):
    """Fold LN gamma/beta + biases into weights; build tables. Returns in_maps."""
    import ml_dtypes

    x = np.asarray(inputs["input_tensor"], np.float32)  # (B, T, E)
    mask = np.asarray(inputs["sequence_mask"])  # (B, T) bool
    gamma = np.asarray(inputs["ln_scale"], np.float32)
    beta = np.asarray(inputs["ln_bias"], np.float32)
    q_w = np.asarray(inputs["q_w"], np.float32)
    k_w = np.asarray(inputs["k_w"], np.float32)
    v_w = np.asarray(inputs["v_w"], np.float32)
    in_b = np.asarray(inputs["in_proj_bias"], np.float32)
    out_w = np.asarray(inputs["out_w"], np.float32)
    out_b = np.asarray(inputs["out_b"], np.float32)
    pos_u = np.asarray(inputs["pos_bias_u"], np.float32).reshape(-1)
    pos_v = np.asarray(inputs["pos_bias_v"], np.float32).reshape(-1)

    bias_k, bias_q, bias_v = np.split(in_b, 3)  # torch unpack order

    qw_eff = q_w * gamma[None, :]
    kw_eff = k_w * gamma[None, :]
    vw_eff = v_w * gamma[None, :]
    bias_qu = q_w @ beta + bias_q + pos_u
    bias_qv = q_w @ beta + bias_q + pos_v
    bias_v_full = v_w @ beta + bias_v
    out_b_eff = out_b + out_w @ bias_v_full

    qwT = np.ascontiguousarray(qw_eff.T).astype(ml_dtypes.bfloat16)
    kwT = np.ascontiguousarray(kw_eff.T).astype(ml_dtypes.bfloat16)
    vwT = np.ascontiguousarray(vw_eff.T).astype(ml_dtypes.bfloat16)
    owT = np.ascontiguousarray(out_w.T).astype(ml_dtypes.bfloat16)

    pbias = np.zeros((128, 8), np.float32)
    for et in range(NC):
        pbias[:, et] = bias_qu[et * 128 : (et + 1) * 128]
        pbias[:, 4 + et] = bias_qv[et * 128 : (et + 1) * 128]

    inv_freq = 1.0 / (10000.0 ** (np.arange(0, DH, 2, dtype=np.float64) / DH))
    ang = np.outer(inv_freq, np.arange(T, dtype=np.float64))  # (32, T)
    cosT = np.cos(ang)
    sinT = np.sin(ang)
    tabk = np.concatenate([cosT, sinT], axis=0).astype(np.float32)  # (64, T)
    tblsin = np.tile(sinT, (4, 1)).astype(np.float32)  # (128, T)
    tblcos = np.tile(cosT, (4, 1)).astype(np.float32)  # (128, T)

    obias = np.tile(out_b_eff.reshape(1, E), (128, 1)).astype(np.float32)
    ident = np.eye(128, dtype=np.float32)

    # Mc: pc[:,m] = sum_k mc[k,m] C[k]; per 64-block: out[0:32]=C[32:64],
    # out[32:64]=-C[0:32]
    mc = np.zeros((128, 128), np.float32)
    for base in (0, 64):
        for f in range(32):
            mc[base + 32 + f, base + f] = 1.0
            mc[base + f, base + 32 + f] = -1.0

    shared = {
        "qwT": qwT,
        "kwT": kwT,
        "vwT": vwT,
        "owT": owT,
        "pbias": pbias,
        "tblsin": tblsin,
        "tblcos": tblcos,
        "tabk": tabk,
        "obias": obias,
        "ident": ident,
        "mcomb": mc,
    }
    in_maps = []
    for b in range(N_CORES):
        mt = np.zeros((128, NT), np.float32)
        mb = mask[b].astype(np.float32)
        for jt in range(NT):
            mt[:, jt] = mb[jt * 128 : (jt + 1) * 128]
        in_maps.append({"x": np.ascontiguousarray(x[b]), "maskt": mt, **shared})
    return in_maps


def kernel(**inputs) -> np.ndarray:
    from concourse.bass_utils import run_bass_kernel_spmd

    in_maps = _host_prep(inputs)
    if "nc" not in _CACHE:
        _CACHE["nc"] = _build_nc()
    trace = os.environ.get("KERNEL_TRACE", "0") == "1"
    res = run_bass_kernel_spmd(
        _CACHE["nc"], in_maps, core_ids=list(range(N_CORES)), trace=trace
    )
    _CACHE["last_result"] = res
    out = np.stack([res.results[i]["out"] for i in range(N_CORES)], axis=0)
    return out.astype(np.float32)
